# revision 4
# baseline (speedup 1.0000x reference)
"""Expert-parallel MoE kernel for Trainium2 (8 NeuronCores), v2.

Strategy (matches the expert-parallel sharding hint):
  - Router evaluated on host with the exact jax ops of the reference, so
    top-k decisions match bit-for-bit.
  - Tokens are gathered per expert on host; each of the 8 cores owns one
    expert's weights and runs the fused MLP
        Y = (silu(X @ G^T) * (X @ U^T)) @ D^T
    in bf16 with fp32 PSUM accumulation.
  - Outputs are combined on host: out[token] += mean_w[e] * Y_e[row].

v2 schedule (vs v1): token capacity C is processed in two halves so the
G/U weight stream runs twice (32 MB) instead of once per 512-token tile
(80 MB); D^T stays SBUF-resident, split into two H-halves whose re-loads
overlap the opposite half's compute; the intermediate Hh stays on-chip
per half; output is written bf16 in H-major (y^T) form.
"""

import sys
from contextlib import ExitStack

if "/opt/trn_rl_repo" not in sys.path:
    sys.path.insert(0, "/opt/trn_rl_repo")

import ml_dtypes
import numpy as np

import concourse.bacc as bacc
import concourse.mybir as mybir
import concourse.tile as tile
from concourse.bass_utils import run_bass_kernel_spmd

B, S, H, I, E, TOPK = 4, 2048, 1024, 4096, 8, 2
T = B * S
KCH = H // 128   # 8 contraction chunks over H
IB = I // 128    # 32 blocks over I
HB = H // 128    # 8 output blocks over H
BF16 = mybir.dt.bfloat16
F32 = mybir.dt.float32

_prog_cache: dict[tuple, object] = {}


def _ctiles(C):
    """Free-dim tiles of 512 (tail multiple of 128) covering [0, C)."""
    out = []
    c = 0
    while c < C:
        s = min(512, C - c)
        out.append((c, s))
        c += s
    return out


def _halves(C):
    """Split the ctile list into two halves (weights stream once per half)."""
    tiles = _ctiles(C)
    k = len(tiles) // 2
    return tiles[:k], tiles[k:]


def build_program(C, reps=1):
    key = (C, reps)
    if key in _prog_cache:
        return _prog_cache[key]
    nc = bacc.Bacc("TRN2", target_bir_lowering=False, debug=False, num_devices=8)

    xt_d = nc.dram_tensor("xt", [128, KCH, C], BF16, kind="ExternalInput").ap()
    gt_d = nc.dram_tensor("gt", [IB, 128, KCH, 128], BF16, kind="ExternalInput").ap()
    ut_d = nc.dram_tensor("ut", [IB, 128, KCH, 128], BF16, kind="ExternalInput").ap()
    # D^T packed [128(p over i%128), IB, H]; split along H for double buffering
    dt_d = nc.dram_tensor("dt", [128, IB, H], BF16, kind="ExternalInput").ap()
    # y^T blocks: y_d[hb] = Y^T[hb*128:(hb+1)*128, :]
    y_d = nc.dram_tensor("y", [HB, 128, C], BF16, kind="ExternalOutput").ap()

    with tile.TileContext(nc) as tc:
        with ExitStack() as stack:
            if reps > 1:
                stack.enter_context(tc.For_i(0, reps, 1))
            _emit_body(nc, tc, C, xt_d, gt_d, ut_d, dt_d, y_d)

    nc.compile()
    _prog_cache[key] = nc
    return nc


def _emit_body(nc, tc, C, xt_d, gt_d, ut_d, dt_d, y_d):
    halves = _halves(C)
    max_half = max(sum(cs for _, cs in h) for h in halves)

    with (
        tc.tile_pool(name="wpool", bufs=3) as wpool,
        tc.tile_pool(name="xpool", bufs=1) as xpool,
        tc.tile_pool(name="dpool", bufs=1) as dpool,
        tc.tile_pool(name="hpool", bufs=1) as hpool,
        tc.tile_pool(name="spool", bufs=2) as spool,
        tc.tile_pool(name="ypool", bufs=2) as ypool,
        tc.tile_pool(name="psum", bufs=1, space="PSUM") as psum,
    ):
        # X^T resident for the full run: [128, KCH, C] (34.8 KB/partition)
        xs = xpool.tile([128, KCH, C], BF16, tag="xt")
        nc.sync.dma_start(xs[:], xt_d)
        # D^T resident in two H-halves (32 KB/partition each)
        dta = dpool.tile([128, IB, H // 2], BF16, tag="dta")
        nc.sync.dma_start(dta[:], dt_d[:, :, : H // 2])
        dtb = dpool.tile([128, IB, H // 2], BF16, tag="dtb")
        nc.sync.dma_start(dtb[:], dt_d[:, :, H // 2 :])

        hhs = []
        for ib in range(IB):
            hh = hpool.tile([128, max_half], BF16, tag=f"hh{ib}", name=f"hh{ib}")
            hhs.append(hh)

        for tiles in halves:
            base = tiles[0][0]
            # ---- stage 1: Hh[i, c] = silu(G X^T) * (U X^T), i-major ----
            for ib in range(IB):
                gt = wpool.tile([128, KCH, 128], BF16, tag="gt")
                nc.sync.dma_start(gt[:], gt_d[ib])
                ut = wpool.tile([128, KCH, 128], BF16, tag="ut")
                nc.sync.dma_start(ut[:], ut_d[ib])
                sls = {}
                # process ct tiles in pairs so each stationary weight chunk
                # is reused across two moving tiles (halves LDWEIGHTS count)
                for w, wt, atag in ((0, gt, "a1"), (1, ut, "a2")):
                    for p0 in range(0, len(tiles), 2):
                        grp = list(enumerate(tiles))[p0 : p0 + 2]
                        accs = []
                        for ci, (c0, cs) in grp:
                            acc = psum.tile(
                                [128, cs], F32, tag=f"{atag}{ci % 2}", name="acc"
                            )
                            accs.append(acc)
                        for k in range(KCH):
                            for gi, (ci, (c0, cs)) in enumerate(grp):
                                nc.tensor.matmul(
                                    accs[gi][:], wt[:, k, :],
                                    xs[:, k, c0 : c0 + cs],
                                    start=(k == 0), stop=(k == KCH - 1),
                                )
                        if w == 0:
                            for gi, (ci, (c0, cs)) in enumerate(grp):
                                sl = spool.tile(
                                    [128, cs], F32, tag=f"sl{ci % 2}", name="sl"
                                )
                                nc.scalar.activation(
                                    sl[:], accs[gi][:],
                                    mybir.ActivationFunctionType.Silu,
                                )
                                sls[ci] = sl
                        else:
                            for gi, (ci, (c0, cs)) in enumerate(grp):
                                lc0 = c0 - base
                                nc.vector.tensor_mul(
                                    hhs[ib][:, lc0 : lc0 + cs],
                                    sls[ci][:], accs[gi][:],
                                )

            # ---- stage 2: Y^T[h, c] = D Hh, two H-passes over resident D^T ----
            for hpass, dts in ((0, dta), (1, dtb)):
                for c0, cs in tiles:
                    lc0 = c0 - base
                    pys = [
                        psum.tile([128, cs], F32, tag=f"y{j}", name=f"py{j}")
                        for j in range(4)
                    ]
                    for ic in range(IB):
                        for j in range(4):
                            nc.tensor.matmul(
                                pys[j][:],
                                dts[:, ic, j * 128 : (j + 1) * 128],
                                hhs[ic][:, lc0 : lc0 + cs],
                                start=(ic == 0), stop=(ic == IB - 1),
                            )
                    for j in range(4):
                        hb = hpass * 4 + j
                        yt = ypool.tile([128, cs], BF16, tag=f"yt{j}", name="yt")
                        nc.scalar.copy(yt[:], pys[j][:])
                        nc.sync.dma_start(y_d[hb][:, c0 : c0 + cs], yt[:])


def _routing(x, router_w):
    """Replicate the reference's routing decisions with identical jax ops."""
    import jax
    import jax.numpy as jnp

    xf = jnp.asarray(x).reshape(-1, H)
    logits = xf @ jnp.asarray(router_w).T
    probs = jax.nn.softmax(logits, axis=-1)
    topk_p, topk_i = jax.lax.top_k(probs, TOPK)
    topk_p = topk_p / topk_p.sum(axis=-1, keepdims=True)
    return np.asarray(topk_p), np.asarray(topk_i)


def prepare(x, router_w, gate_w, up_w, down_w):
    """Host-side dispatch: returns (nc, in_maps, combine)."""
    topk_p, topk_i = _routing(x, router_w)
    xf = np.ascontiguousarray(np.asarray(x, dtype=np.float32).reshape(T, H))

    idxs, weights = [], []
    for e in range(E):
        sel = topk_i == e
        mask = sel.any(axis=-1)
        w_tok = (topk_p * sel).sum(axis=-1)
        cnt = int(mask.sum())
        mean_w = float(w_tok.sum() / max(cnt, 1)) if cnt > 0 else 0.0
        idxs.append(np.nonzero(mask)[0])
        weights.append(np.float32(mean_w))

    cmax = max(len(ix) for ix in idxs)
    C = ((cmax + 127) // 128) * 128

    xf_bf = xf.astype(ml_dtypes.bfloat16)
    in_maps = []
    for e in range(E):
        ix = idxs[e]
        # X^T packed [128(p), KCH(k), C]: partition p of chunk k holds row
        # h = k*128 + p
        xt = np.zeros((128, KCH, C), dtype=ml_dtypes.bfloat16)
        xt[:, :, : len(ix)] = (
            xf_bf[ix].T.reshape(KCH, 128, len(ix)).transpose(1, 0, 2)
        )
        # G^T is [H, I]; packed [IB, 128(p), KCH(k), 128(i)] with h = k*128+p
        gT = np.asarray(gate_w[e], dtype=np.float32).T.astype(ml_dtypes.bfloat16)
        uT = np.asarray(up_w[e], dtype=np.float32).T.astype(ml_dtypes.bfloat16)
        gt = np.ascontiguousarray(
            gT.reshape(KCH, 128, IB, 128).transpose(2, 1, 0, 3)
        )
        ut = np.ascontiguousarray(
            uT.reshape(KCH, 128, IB, 128).transpose(2, 1, 0, 3)
        )
        # D^T is [I, H]; packed [128(p over i%128), IB(ic), H] with i = ic*128+p
        dT = np.asarray(down_w[e], dtype=np.float32).T.astype(ml_dtypes.bfloat16)
        dt = np.ascontiguousarray(dT.reshape(IB, 128, H).transpose(1, 0, 2))
        in_maps.append({"xt": xt, "gt": gt, "ut": ut, "dt": dt})

    nc = build_program(C)

    def combine(results):
        out = np.zeros((T, H), dtype=np.float32)
        for e in range(E):
            ix = idxs[e]
            # y is [HB, 128, C] bf16 blocks of Y^T
            yT = results[e]["y"].reshape(H, C).astype(np.float32)
            out[ix] += weights[e] * yT[:, : len(ix)].T
        return out.reshape(B, S, H)

    return nc, in_maps, combine


def kernel(x, router_w, gate_w, up_w, down_w):
    nc, in_maps, combine = prepare(x, router_w, gate_w, up_w, down_w)
    res = run_bass_kernel_spmd(nc, in_maps, list(range(8)))
    return combine(res.results)


# revision 5
# speedup vs baseline: 1.0770x; 1.0770x over previous
"""Expert-parallel MoE kernel for Trainium2 (8 NeuronCores), v2.

Strategy (matches the expert-parallel sharding hint):
  - Router evaluated on host with the exact jax ops of the reference, so
    top-k decisions match bit-for-bit.
  - Tokens are gathered per expert on host; each of the 8 cores owns one
    expert's weights and runs the fused MLP
        Y = (silu(X @ G^T) * (X @ U^T)) @ D^T
    in bf16 with fp32 PSUM accumulation.
  - Outputs are combined on host: out[token] += mean_w[e] * Y_e[row].

v2 schedule (vs v1): token capacity C is processed in two halves so the
G/U weight stream runs twice (32 MB) instead of once per 512-token tile
(80 MB); D^T stays SBUF-resident, split into two H-halves whose re-loads
overlap the opposite half's compute; the intermediate Hh stays on-chip
per half; output is written bf16 in H-major (y^T) form.
"""

import sys
from contextlib import ExitStack

if "/opt/trn_rl_repo" not in sys.path:
    sys.path.insert(0, "/opt/trn_rl_repo")

import ml_dtypes
import numpy as np

import concourse.bacc as bacc
import concourse.mybir as mybir
import concourse.tile as tile
from concourse.bass_utils import run_bass_kernel_spmd

B, S, H, I, E, TOPK = 4, 2048, 1024, 4096, 8, 2
T = B * S
KCH = H // 128   # 8 contraction chunks over H
IB = I // 128    # 32 blocks over I
HB = H // 128    # 8 output blocks over H
BF16 = mybir.dt.bfloat16
F32 = mybir.dt.float32

_prog_cache: dict[tuple, object] = {}


def _ctiles(C):
    """Free-dim tiles of 512 (tail multiple of 128) covering [0, C)."""
    out = []
    c = 0
    while c < C:
        s = min(512, C - c)
        out.append((c, s))
        c += s
    return out


def _halves(C):
    """Split the ctile list into two halves (weights stream once per half)."""
    tiles = _ctiles(C)
    k = len(tiles) // 2
    return tiles[:k], tiles[k:]


def build_program(C, reps=1):
    key = (C, reps)
    if key in _prog_cache:
        return _prog_cache[key]
    nc = bacc.Bacc("TRN2", target_bir_lowering=False, debug=False, num_devices=8)

    xt_d = nc.dram_tensor("xt", [128, KCH, C], BF16, kind="ExternalInput").ap()
    gt_d = nc.dram_tensor("gt", [IB, 128, KCH, 128], BF16, kind="ExternalInput").ap()
    ut_d = nc.dram_tensor("ut", [IB, 128, KCH, 128], BF16, kind="ExternalInput").ap()
    # D^T packed [128(p over i%128), IB, H]; split along H for double buffering
    dt_d = nc.dram_tensor("dt", [128, IB, H], BF16, kind="ExternalInput").ap()
    # y^T blocks: y_d[hb] = Y^T[hb*128:(hb+1)*128, :]
    y_d = nc.dram_tensor("y", [HB, 128, C], BF16, kind="ExternalOutput").ap()

    with tile.TileContext(nc) as tc:
        with ExitStack() as stack:
            if reps > 1:
                stack.enter_context(tc.For_i(0, reps, 1))
            _emit_body(nc, tc, C, xt_d, gt_d, ut_d, dt_d, y_d)

    nc.compile()
    _prog_cache[key] = nc
    return nc


def _emit_body(nc, tc, C, xt_d, gt_d, ut_d, dt_d, y_d):
    halves = _halves(C)
    max_half = max(sum(cs for _, cs in h) for h in halves)

    with (
        tc.tile_pool(name="wpool", bufs=3) as wpool,
        tc.tile_pool(name="xpool", bufs=1) as xpool,
        tc.tile_pool(name="dpool", bufs=1) as dpool,
        tc.tile_pool(name="hpool", bufs=1) as hpool,
        tc.tile_pool(name="spool", bufs=2) as spool,
        tc.tile_pool(name="ypool", bufs=2) as ypool,
        tc.tile_pool(name="psum", bufs=1, space="PSUM") as psum,
    ):
        # X^T resident for the full run: [128, KCH, C] (34.8 KB/partition)
        xs = xpool.tile([128, KCH, C], BF16, tag="xt")
        nc.sync.dma_start(xs[:], xt_d)
        # D^T resident in two H-halves (32 KB/partition each)
        dta = dpool.tile([128, IB, H // 2], BF16, tag="dta")
        nc.sync.dma_start(dta[:], dt_d[:, :, : H // 2])
        dtb = dpool.tile([128, IB, H // 2], BF16, tag="dtb")
        nc.sync.dma_start(dtb[:], dt_d[:, :, H // 2 :])

        hhs = []
        for ib in range(IB):
            hh = hpool.tile([128, max_half], BF16, tag=f"hh{ib}", name=f"hh{ib}")
            hhs.append(hh)

        for tiles in halves:
            base = tiles[0][0]
            # ---- stage 1: Hh[i, c] = silu(G X^T) * (U X^T), i-major ----
            for ib in range(IB):
                gt = wpool.tile([128, KCH, 128], BF16, tag="gt")
                nc.sync.dma_start(gt[:], gt_d[ib])
                ut = wpool.tile([128, KCH, 128], BF16, tag="ut")
                nc.sync.dma_start(ut[:], ut_d[ib])
                for ci, (c0, cs) in enumerate(tiles):
                    a1 = psum.tile([128, cs], F32, tag=f"a1{ci % 2}", name="a1")
                    for k in range(KCH):
                        nc.tensor.matmul(
                            a1[:], gt[:, k, :], xs[:, k, c0 : c0 + cs],
                            start=(k == 0), stop=(k == KCH - 1),
                        )
                    a2 = psum.tile([128, cs], F32, tag=f"a2{ci % 2}", name="a2")
                    for k in range(KCH):
                        nc.tensor.matmul(
                            a2[:], ut[:, k, :], xs[:, k, c0 : c0 + cs],
                            start=(k == 0), stop=(k == KCH - 1),
                        )
                    sl = spool.tile([128, cs], F32, tag=f"sl{ci % 2}", name="sl")
                    nc.scalar.activation(
                        sl[:], a1[:], mybir.ActivationFunctionType.Silu
                    )
                    lc0 = c0 - base
                    nc.vector.tensor_mul(
                        hhs[ib][:, lc0 : lc0 + cs], sl[:], a2[:]
                    )

            # ---- stage 2: Y^T[h, c] = D Hh, two H-passes over resident D^T ----
            for hpass, dts in ((0, dta), (1, dtb)):
                for c0, cs in tiles:
                    lc0 = c0 - base
                    pys = [
                        psum.tile([128, cs], F32, tag=f"y{j}", name=f"py{j}")
                        for j in range(4)
                    ]
                    for ic in range(IB):
                        for j in range(4):
                            nc.tensor.matmul(
                                pys[j][:],
                                dts[:, ic, j * 128 : (j + 1) * 128],
                                hhs[ic][:, lc0 : lc0 + cs],
                                start=(ic == 0), stop=(ic == IB - 1),
                            )
                    for j in range(4):
                        hb = hpass * 4 + j
                        yt = ypool.tile([128, cs], BF16, tag=f"yt{j}", name="yt")
                        nc.scalar.copy(yt[:], pys[j][:])
                        nc.sync.dma_start(y_d[hb][:, c0 : c0 + cs], yt[:])


def _routing(x, router_w):
    """Replicate the reference's routing decisions with identical jax ops."""
    import jax
    import jax.numpy as jnp

    xf = jnp.asarray(x).reshape(-1, H)
    logits = xf @ jnp.asarray(router_w).T
    probs = jax.nn.softmax(logits, axis=-1)
    topk_p, topk_i = jax.lax.top_k(probs, TOPK)
    topk_p = topk_p / topk_p.sum(axis=-1, keepdims=True)
    return np.asarray(topk_p), np.asarray(topk_i)


def prepare(x, router_w, gate_w, up_w, down_w):
    """Host-side dispatch: returns (nc, in_maps, combine)."""
    topk_p, topk_i = _routing(x, router_w)
    xf = np.ascontiguousarray(np.asarray(x, dtype=np.float32).reshape(T, H))

    idxs, weights = [], []
    for e in range(E):
        sel = topk_i == e
        mask = sel.any(axis=-1)
        w_tok = (topk_p * sel).sum(axis=-1)
        cnt = int(mask.sum())
        mean_w = float(w_tok.sum() / max(cnt, 1)) if cnt > 0 else 0.0
        idxs.append(np.nonzero(mask)[0])
        weights.append(np.float32(mean_w))

    cmax = max(len(ix) for ix in idxs)
    C = ((cmax + 127) // 128) * 128

    xf_bf = xf.astype(ml_dtypes.bfloat16)
    in_maps = []
    for e in range(E):
        ix = idxs[e]
        # X^T packed [128(p), KCH(k), C]: partition p of chunk k holds row
        # h = k*128 + p
        xt = np.zeros((128, KCH, C), dtype=ml_dtypes.bfloat16)
        xt[:, :, : len(ix)] = (
            xf_bf[ix].T.reshape(KCH, 128, len(ix)).transpose(1, 0, 2)
        )
        # G^T is [H, I]; packed [IB, 128(p), KCH(k), 128(i)] with h = k*128+p
        gT = np.asarray(gate_w[e], dtype=np.float32).T.astype(ml_dtypes.bfloat16)
        uT = np.asarray(up_w[e], dtype=np.float32).T.astype(ml_dtypes.bfloat16)
        gt = np.ascontiguousarray(
            gT.reshape(KCH, 128, IB, 128).transpose(2, 1, 0, 3)
        )
        ut = np.ascontiguousarray(
            uT.reshape(KCH, 128, IB, 128).transpose(2, 1, 0, 3)
        )
        # D^T is [I, H]; packed [128(p over i%128), IB(ic), H] with i = ic*128+p
        dT = np.asarray(down_w[e], dtype=np.float32).T.astype(ml_dtypes.bfloat16)
        dt = np.ascontiguousarray(dT.reshape(IB, 128, H).transpose(1, 0, 2))
        in_maps.append({"xt": xt, "gt": gt, "ut": ut, "dt": dt})

    nc = build_program(C)

    def combine(results):
        out = np.zeros((T, H), dtype=np.float32)
        for e in range(E):
            ix = idxs[e]
            # y is [HB, 128, C] bf16 blocks of Y^T
            yT = results[e]["y"].reshape(H, C).astype(np.float32)
            out[ix] += weights[e] * yT[:, : len(ix)].T
        return out.reshape(B, S, H)

    return nc, in_maps, combine


def kernel(x, router_w, gate_w, up_w, down_w):
    nc, in_maps, combine = prepare(x, router_w, gate_w, up_w, down_w)
    res = run_bass_kernel_spmd(nc, in_maps, list(range(8)))
    return combine(res.results)
